# revision 30
# baseline (speedup 1.0000x reference)
"""Localized embedding layer (separable 5x5 Gaussian stencil) on 8 trn2 cores.

Math: out[i,j,:] = sum_{di,dj} w(di)w(dj) H[i+di,j+dj,:] / (ri(i)*rj(j))
with w(d) = exp(-c*d^2), c = TILE^2/(2 sigma^2); ri/rj = sums of the valid
taps actually applied (the +-2 i-taps, weight w2 ~ 4.4e-5, are dropped — a
~1e-4 relative contribution, far below bf16 rounding noise).

v3 design (bf16 end-to-end, DMA-roofline oriented):
  - All HBM traffic in bf16 (host converts): ~18 MB/core vs 36 MB f32.
  - Input per core: one SBUF tile X[128p(j%128), 36 row, 2 half, 512 d],
    loaded as 18 row-pair DMAs (SP sequencer).
  - i-conv: v = x[i+1]+x[i+3] as paired DVE tensor_tensor (bf16 2x mode);
    tap weights folded into the j-conv matmuls.
  - j-conv: 2 PSUM-accumulated bf16 matmuls per half with Toeplitz lhsT:
    ps = w1*T@v + T@x[i+2];  T[m,k] = w(m-k)/w_full.
  - Act engine: PSUM->SBUF copy with per-row scale 1/ri, bf16 out.
  - Out DMAs (row pairs) issued from Pool (SWDGE), all gated behind a dummy
    Pool op that depends on the last input pair: input transfers own the
    DMA engines first (no in/out interleaving), then outputs drain
    back-to-back from deep (OB_BUFS) SBUF buffering.
  - Columns j in {0,1,126..129,254,255} (grid edge + half-boundary) are
    recomputed by a strip pass: per output column, 3-5 PSUM-accumulated
    matmuls with scaled i-conv lhsT (j-comb folded into the lhsT scale —
    convolutions commute), one DVE copy applying 1/ri, 4 small SP DMAs.
    Strip work is interleaved into main-loop pairs 2..9.
"""

import sys
import numpy as np

if "/opt/trn_rl_repo" not in sys.path:
    sys.path.insert(0, "/opt/trn_rl_repo")

G = 256          # grid side
D = 512          # feature dim
P = 2            # grid_step halo
NC = 8           # cores
RPC = G // NC    # rows per core = 32
NR = RPC + 2 * P  # input rows per core = 36
TILE = 448.0
SIGMA = 200.0

_cache = {}

# tuning knobs (A/B tested via TimelineSim)
OB_BUFS = 4        # output row-quad tiles buffered in SBUF
HOLD_OUT = True    # gate out DMAs behind the GATE_PAIR-th input pair
GATE_PAIR = 17     # out DMAs wait for this input pair (17 = last)
WARMUP_MM = 14     # dummy PE matmuls to ramp the p-state at kernel start


def _weights():
    c = TILE * TILE / (2.0 * SIGMA * SIGMA)
    return np.exp(-c * np.arange(-P, P + 1) ** 2)   # [w2,w1,1,w1,w2] f64


def _r_vec(drop2=False):
    """r(i) = sum of valid 1D taps at row i (same for columns).

    drop2: exclude the +-2 taps — used for the i-dimension, where the kernel
    skips those taps; the normalizer must match the taps actually applied.
    """
    w = _weights()
    r = np.zeros(G)
    for d in range(-P, P + 1):
        if drop2 and abs(d) == 2:
            continue
        lo, hi = max(0, -d), min(G, G - d)
        r[lo:hi] += w[d + P]
    return r


def _strip_plan():
    """Per fix column c (FS order [j0,j1,j126..j129,j254,j255]): list of
    (xs_col, lhsT_idx). lhsT_idx = 3*denom_group + |d|, denom group
    0 -> w_full, 1 -> r(0), 2 -> r(1). xs cols: 0..3 = j 0..3,
    4..11 = j 124..131, 12..15 = j 252..255."""
    plan = [
        [(0, 3 + 0), (1, 3 + 1), (2, 3 + 2)],                       # j0   /r0
        [(0, 6 + 1), (1, 6 + 0), (2, 6 + 1), (3, 6 + 2)],           # j1   /r1
        [(4, 2), (5, 1), (6, 0), (7, 1), (8, 2)],                   # j126 /wf
        [(5, 2), (6, 1), (7, 0), (8, 1), (9, 2)],                   # j127 /wf
        [(6, 2), (7, 1), (8, 0), (9, 1), (10, 2)],                  # j128 /wf
        [(7, 2), (8, 1), (9, 0), (10, 1), (11, 2)],                 # j129 /wf
        [(12, 6 + 2), (13, 6 + 1), (14, 6 + 0), (15, 6 + 1)],       # j254 /r1
        [(13, 3 + 2), (14, 3 + 1), (15, 3 + 0)],                    # j255 /r0
    ]
    return plan


def _host_consts():
    import ml_dtypes

    bf16 = ml_dtypes.bfloat16
    w = _weights()
    ri = _r_vec(drop2=True)      # i-dim normalizer
    rj = _r_vec()                # j-dim normalizer (all 5 taps)
    w_full = w.sum()
    w1 = float(w[1])
    # Toeplitz block T[m, k] = w(m-k)/w_full, |m-k| <= 2 (interior j columns;
    # edge/boundary columns are recomputed by the strip pass). lhsT = T.
    T = np.zeros((128, 128))
    for d in range(-P, P + 1):
        for m in range(128):
            k = m + d
            if 0 <= k < 128:
                T[k, m] = w[d + P] / w_full
    wt = np.zeros((128, 2, 128), dtype=np.float64)
    wt[:, 0, :] = T
    wt[:, 1, :] = w1 * T
    wt = wt.astype(bf16)
    # strip i-conv lhsT base [36, 32]: (Wb @ xs_col)[i] = sum_{k=1..3} w[k]
    # * xs[i+k]; 9 scaled variants fold the j-comb weight / column norm in.
    Wb = np.zeros((NR, RPC))
    for i in range(RPC):
        for k in range(1, 4):
            Wb[i + k, i] = w[k]
    denoms = [w_full, rj[0], rj[1]]
    taps = [w[2], w[3], w[4]]    # w0, w1, w2
    ws9 = np.zeros((NR, 9, RPC), dtype=np.float64)
    for g in range(3):
        for t in range(3):
            ws9[:, 3 * g + t, :] = (taps[t] / denoms[g]) * Wb
    ws9 = ws9.astype(bf16)
    # per-core row scales: st[:, i] = 1/ri(global_row); sfix same, [32,1]
    sts, sfixes = [], []
    for c in range(NC):
        s = (1.0 / ri[RPC * c: RPC * (c + 1)]).astype(np.float32)
        sts.append(np.broadcast_to(s[None, :], (128, RPC)).copy())
        sf = np.zeros((RPC, 1), dtype=np.float32)
        sf[:, 0] = s
        sfixes.append(sf)
    return wt, ws9, sts, sfixes


def _build_nc():
    import concourse.bass as bass  # noqa: F401
    import concourse.mybir as mybir
    import concourse.tile as tile
    from concourse import bacc

    f32 = mybir.dt.float32
    bf16 = mybir.dt.bfloat16
    add = mybir.AluOpType.add

    nc = bacc.Bacc(None, target_bir_lowering=False, debug=False)
    x_dram = nc.declare_dram_parameter("x", [NR, 128, 2, D], bf16, isOutput=False)
    wt_dram = nc.declare_dram_parameter("wt", [128, 2, 128], bf16, isOutput=False)
    w9_dram = nc.declare_dram_parameter("ws9", [NR, 9, RPC], bf16, isOutput=False)
    st_dram = nc.declare_dram_parameter("st", [128, RPC], f32, isOutput=False)
    sf_dram = nc.declare_dram_parameter("sfix", [RPC, 1], f32, isOutput=False)
    y_dram = nc.declare_dram_parameter("y", [RPC, 128, 2, D], bf16, isOutput=True)

    plan = _strip_plan()

    with tile.TileContext(nc) as tc:
        with (
            tc.tile_pool(name="const", bufs=1) as cpool,
            tc.tile_pool(name="x", bufs=1) as xpool,
            tc.tile_pool(name="uv", bufs=3) as tpool,
            tc.tile_pool(name="out", bufs=OB_BUFS) as opool,
            tc.tile_pool(name="fix", bufs=1) as fpool,
            tc.tile_pool(name="psum", bufs=3, space="PSUM") as ppool,
            tc.tile_pool(name="psfix", bufs=2, space="PSUM") as pfpool,
        ):
            # ---- PE warm-up: dummy matmuls on an uninitialized tile (no
            # input deps -> run at t~0 back-to-back) ramp the tensor engine
            # to full clock before the first real matmul arrives; their
            # garbage PSUM output is overwritten by start=True accumulations
            wu = cpool.tile([128, 512], bf16)
            nc.gpsimd.memset(wu[:], 0.0)
            for _ in range(WARMUP_MM):
                psw = pfpool.tile([RPC, D], f32, tag="psf")
                nc.tensor.matmul(psw[:], wu[:, 0:RPC], wu[:], start=True, stop=True)

            # ---- input: one big X tile, row-pair DMAs (first = immediate) --
            X = xpool.tile([128, NR, 2, D], bf16)

            def load_pair(q):
                nc.sync.dma_start(
                    X[:, 2 * q:2 * q + 2, :, :],
                    x_dram[2 * q:2 * q + 2].rearrange("r p h d -> p r h d"),
                )

            for q in range(3):
                load_pair(q)

            wtt = cpool.tile([128, 2, 128], bf16)
            nc.sync.dma_start(wtt[:], wt_dram[:])
            w9t = cpool.tile([NR, 9, RPC], bf16)
            nc.sync.dma_start(w9t[:], w9_dram[:])
            stt = cpool.tile([128, RPC], f32)
            nc.sync.dma_start(stt[:], st_dram[:])
            sft = cpool.tile([RPC, 1], f32)
            nc.sync.dma_start(sft[:], sf_dram[:])

            # strip inputs: xs cols 0..3 = j 0..3 | 4..11 = j 124..131 |
            # 12..15 = j 252..255
            xs = fpool.tile([NR, 16, D], bf16, tag="xs")
            nc.sync.dma_start(xs[:, 0:4, :], x_dram[:, 0:4, 0, :])
            nc.sync.dma_start(xs[:, 4:8, :], x_dram[:, 124:128, 0, :])
            nc.sync.dma_start(xs[:, 8:12, :], x_dram[:, 0:4, 1, :])
            nc.sync.dma_start(xs[:, 12:16, :], x_dram[:, 124:128, 1, :])

            for q in range(3, NR // 2):
                load_pair(q)

            FS = fpool.tile([RPC, 8, D], bf16, tag="FS")

            if HOLD_OUT:
                # tiny Pool op reading the GATE_PAIR-th input pair: all
                # Pool-issued out DMAs queue behind it (in-order sequencer),
                # so input transfers own the DMA engines while streaming
                gate = cpool.tile([128, 8], bf16)
                nc.gpsimd.tensor_copy(gate[:], X[:, 2 * GATE_PAIR + 1, 1, 0:8])

            # ---- main loop: 8 row quads (strip columns interleaved) --------
            # 4 rows per output DMA: SWDGE descriptor-gen cost per byte drops
            # 4x, so the drain is transfer-paced, not Pool-sequencer-paced
            for q4 in range(RPC // 4):
                i0 = 4 * q4
                for pp in (0, 1):
                    p0 = i0 + 2 * pp
                    v = tpool.tile([128, 2, 2, D], bf16, tag="v")
                    nc.vector.tensor_tensor(
                        v[:], X[:, p0 + 1:p0 + 3, :, :],
                        X[:, p0 + 3:p0 + 5, :, :], add)
                    if pp == 0:
                        ob = opool.tile([128, 4, 2, D], bf16, tag="ob")
                    for rr in (0, 1):
                        i = p0 + rr
                        ps = ppool.tile([128, 2, D], f32, tag="ps")
                        for h in (0, 1):
                            nc.tensor.matmul(ps[:, h, :], wtt[:, 1, :],
                                             v[:, rr, h, :],
                                             start=True, stop=False)
                            nc.tensor.matmul(ps[:, h, :], wtt[:, 0, :],
                                             X[:, i + 2, h, :],
                                             start=False, stop=True)
                        nc.scalar.mul(ob[:, i - i0, :, :], ps[:], stt[:, i:i + 1])
                (nc.gpsimd if HOLD_OUT else nc.scalar).dma_start(
                    y_dram[i0:i0 + 4, 2:126].rearrange("r p h d -> p r h d"),
                    ob[2:126, :, :, :],
                )
                # strip column q4: PSUM-accumulated scaled-lhsT matmuls +
                # one DVE copy applying 1/ri
                c = q4
                if 0 <= c < 8:
                    psf = pfpool.tile([RPC, D], f32, tag="psf")
                    mm = plan[c]
                    for n, (xc, s) in enumerate(mm):
                        nc.tensor.matmul(psf[:], w9t[:, s, :], xs[:, xc, :],
                                         start=(n == 0), stop=(n == len(mm) - 1))
                    nc.vector.tensor_scalar_mul(
                        FS[:, c:c + 1, :], psf[:], sft[0:RPC, 0:1])

            # fix DMAs from SP (its queue is idle by now; transfers slot in
            # behind the remaining input pairs, ahead of the output drain)
            nc.sync.dma_start(y_dram[:, 0:2, 0, :], FS[:, 0:2, :])
            nc.sync.dma_start(y_dram[:, 126:128, 0, :], FS[:, 2:4, :])
            nc.sync.dma_start(y_dram[:, 0:2, 1, :], FS[:, 4:6, :])
            nc.sync.dma_start(y_dram[:, 126:128, 1, :], FS[:, 6:8, :])
    nc.finalize()
    return nc


def _get_program():
    if "nc" not in _cache:
        _cache["nc"] = _build_nc()
        _cache["consts"] = _host_consts()
    return _cache["nc"], _cache["consts"]


def _make_in_maps(H):
    import ml_dtypes

    bf16 = ml_dtypes.bfloat16
    nc, (wt, ws9, sts, sfixes) = _get_program()
    Hb = np.asarray(H, dtype=np.float32).astype(bf16).reshape(G, G, D)
    Hp = np.zeros((G + 2 * P, G, D), dtype=bf16)
    Hp[P:P + G] = Hb
    in_maps = []
    for c in range(NC):
        shard = Hp[RPC * c: RPC * c + NR]                      # [36, 256, 512]
        shard = shard.reshape(NR, 2, 128, D).transpose(0, 2, 1, 3)
        shard = np.ascontiguousarray(shard)                    # [36, 128, 2, 512]
        in_maps.append(
            {"x": shard, "wt": wt, "ws9": ws9,
             "st": sts[c], "sfix": sfixes[c]}
        )
    return in_maps


def kernel(H, xy=None):
    from concourse.bass_utils import run_bass_kernel_spmd

    nc, _ = _get_program()
    in_maps = _make_in_maps(H)
    res = run_bass_kernel_spmd(nc, in_maps, list(range(NC))).results
    # y [32, 128, 2, 512] bf16 -> [32*256, 512] f32 with j = h*128 + p
    outs = []
    for c in range(NC):
        y = np.asarray(res[c]["y"]).astype(np.float32)
        outs.append(y.transpose(0, 2, 1, 3).reshape(RPC * G, D))
    return np.concatenate(outs, axis=0)


# revision 32
# speedup vs baseline: 1.0355x; 1.0355x over previous
"""Localized embedding layer (separable 5x5 Gaussian stencil) on 8 trn2 cores.

Math: out[i,j,:] = sum_{di,dj} w(di)w(dj) H[i+di,j+dj,:] / (ri(i)*rj(j))
with w(d) = exp(-c*d^2), c = TILE^2/(2 sigma^2); ri/rj = sums of the valid
taps actually applied (the +-2 i-taps, weight w2 ~ 4.4e-5, are dropped — a
~1e-4 relative contribution, far below bf16 rounding noise).

v3 design (bf16 end-to-end, DMA-roofline oriented):
  - All HBM traffic in bf16 (host converts): ~18 MB/core vs 36 MB f32.
  - Input per core: one SBUF tile X[128p(j%128), 36 row, 2 half, 512 d],
    loaded as 18 row-pair DMAs (SP sequencer).
  - i-conv: v = x[i+1]+x[i+3] as paired DVE tensor_tensor (bf16 2x mode);
    tap weights folded into the j-conv matmuls.
  - j-conv: 2 PSUM-accumulated bf16 matmuls per half with Toeplitz lhsT:
    ps = w1*T@v + T@x[i+2];  T[m,k] = w(m-k)/w_full.
  - Act engine: PSUM->SBUF copy with per-row scale 1/ri, bf16 out.
  - Out DMAs (row quads) issued from Pool (SWDGE), all gated behind a
    dummy Pool op that depends on the last input pair: input transfers own
    the DMA engines first (no in/out interleaving), then outputs drain
    back-to-back from deep (OB_BUFS row-quad) SBUF buffering.
  - Columns j in {0,1,126..129,254,255} (grid edge + half-boundary) are
    recomputed by a strip pass: per output column, 3-5 PSUM-accumulated
    matmuls with scaled i-conv lhsT (j-comb folded into the lhsT scale —
    convolutions commute), one DVE copy applying 1/ri, 4 small SP DMAs.
    Strip column c is emitted inside main-loop quad c.
"""

import sys
import numpy as np

if "/opt/trn_rl_repo" not in sys.path:
    sys.path.insert(0, "/opt/trn_rl_repo")

G = 256          # grid side
D = 512          # feature dim
P = 2            # grid_step halo
NC = 8           # cores
RPC = G // NC    # rows per core = 32
NR = RPC + 2 * P  # input rows per core = 36
TILE = 448.0
SIGMA = 200.0

_cache = {}

# tuning knobs (A/B tested via TimelineSim)
OB_BUFS = 6        # output row-quad tiles buffered in SBUF
HOLD_OUT = True    # gate out DMAs behind the GATE_PAIR-th input pair
GATE_PAIR = 17     # out DMAs wait for this input pair (17 = last)
WARMUP_MM = 14     # dummy PE matmuls to ramp the p-state at kernel start


def _weights():
    c = TILE * TILE / (2.0 * SIGMA * SIGMA)
    return np.exp(-c * np.arange(-P, P + 1) ** 2)   # [w2,w1,1,w1,w2] f64


def _r_vec(drop2=False):
    """r(i) = sum of valid 1D taps at row i (same for columns).

    drop2: exclude the +-2 taps — used for the i-dimension, where the kernel
    skips those taps; the normalizer must match the taps actually applied.
    """
    w = _weights()
    r = np.zeros(G)
    for d in range(-P, P + 1):
        if drop2 and abs(d) == 2:
            continue
        lo, hi = max(0, -d), min(G, G - d)
        r[lo:hi] += w[d + P]
    return r


def _strip_plan():
    """Per fix column c (FS order [j0,j1,j126..j129,j254,j255]): list of
    (xs_col, lhsT_idx). lhsT_idx = 3*denom_group + |d|, denom group
    0 -> w_full, 1 -> r(0), 2 -> r(1). xs cols: 0..3 = j 0..3,
    4..11 = j 124..131, 12..15 = j 252..255."""
    plan = [
        [(0, 3 + 0), (1, 3 + 1), (2, 3 + 2)],                       # j0   /r0
        [(0, 6 + 1), (1, 6 + 0), (2, 6 + 1), (3, 6 + 2)],           # j1   /r1
        [(4, 2), (5, 1), (6, 0), (7, 1), (8, 2)],                   # j126 /wf
        [(5, 2), (6, 1), (7, 0), (8, 1), (9, 2)],                   # j127 /wf
        [(6, 2), (7, 1), (8, 0), (9, 1), (10, 2)],                  # j128 /wf
        [(7, 2), (8, 1), (9, 0), (10, 1), (11, 2)],                 # j129 /wf
        [(12, 6 + 2), (13, 6 + 1), (14, 6 + 0), (15, 6 + 1)],       # j254 /r1
        [(13, 3 + 2), (14, 3 + 1), (15, 3 + 0)],                    # j255 /r0
    ]
    return plan


def _host_consts():
    import ml_dtypes

    bf16 = ml_dtypes.bfloat16
    w = _weights()
    ri = _r_vec(drop2=True)      # i-dim normalizer
    rj = _r_vec()                # j-dim normalizer (all 5 taps)
    w_full = w.sum()
    w1 = float(w[1])
    # Toeplitz block T[m, k] = w(m-k)/w_full, |m-k| <= 2 (interior j columns;
    # edge/boundary columns are recomputed by the strip pass). lhsT = T.
    T = np.zeros((128, 128))
    for d in range(-P, P + 1):
        for m in range(128):
            k = m + d
            if 0 <= k < 128:
                T[k, m] = w[d + P] / w_full
    wt = np.zeros((128, 2, 128), dtype=np.float64)
    wt[:, 0, :] = T
    wt[:, 1, :] = w1 * T
    wt = wt.astype(bf16)
    # strip i-conv lhsT base [36, 32]: (Wb @ xs_col)[i] = sum_{k=1..3} w[k]
    # * xs[i+k]; 9 scaled variants fold the j-comb weight / column norm in.
    Wb = np.zeros((NR, RPC))
    for i in range(RPC):
        for k in range(1, 4):
            Wb[i + k, i] = w[k]
    denoms = [w_full, rj[0], rj[1]]
    taps = [w[2], w[3], w[4]]    # w0, w1, w2
    ws9 = np.zeros((NR, 9, RPC), dtype=np.float64)
    for g in range(3):
        for t in range(3):
            ws9[:, 3 * g + t, :] = (taps[t] / denoms[g]) * Wb
    ws9 = ws9.astype(bf16)
    # per-core row scales: st[:, i] = 1/ri(global_row); sfix same, [32,1]
    sts, sfixes = [], []
    for c in range(NC):
        s = (1.0 / ri[RPC * c: RPC * (c + 1)]).astype(np.float32)
        sts.append(np.broadcast_to(s[None, :], (128, RPC)).copy())
        sf = np.zeros((RPC, 1), dtype=np.float32)
        sf[:, 0] = s
        sfixes.append(sf)
    return wt, ws9, sts, sfixes


def _build_nc():
    import concourse.bass as bass  # noqa: F401
    import concourse.mybir as mybir
    import concourse.tile as tile
    from concourse import bacc

    f32 = mybir.dt.float32
    bf16 = mybir.dt.bfloat16
    add = mybir.AluOpType.add

    nc = bacc.Bacc(None, target_bir_lowering=False, debug=False)
    x_dram = nc.declare_dram_parameter("x", [NR, 128, 2, D], bf16, isOutput=False)
    wt_dram = nc.declare_dram_parameter("wt", [128, 2, 128], bf16, isOutput=False)
    w9_dram = nc.declare_dram_parameter("ws9", [NR, 9, RPC], bf16, isOutput=False)
    st_dram = nc.declare_dram_parameter("st", [128, RPC], f32, isOutput=False)
    sf_dram = nc.declare_dram_parameter("sfix", [RPC, 1], f32, isOutput=False)
    y_dram = nc.declare_dram_parameter("y", [RPC, 128, 2, D], bf16, isOutput=True)

    plan = _strip_plan()

    with tile.TileContext(nc) as tc:
        with (
            tc.tile_pool(name="const", bufs=1) as cpool,
            tc.tile_pool(name="x", bufs=1) as xpool,
            tc.tile_pool(name="uv", bufs=3) as tpool,
            tc.tile_pool(name="out", bufs=OB_BUFS) as opool,
            tc.tile_pool(name="fix", bufs=1) as fpool,
            tc.tile_pool(name="psum", bufs=3, space="PSUM") as ppool,
            tc.tile_pool(name="psfix", bufs=2, space="PSUM") as pfpool,
        ):
            # ---- PE warm-up: dummy matmuls on an uninitialized tile (no
            # input deps -> run at t~0 back-to-back) ramp the tensor engine
            # to full clock before the first real matmul arrives; their
            # garbage PSUM output is overwritten by start=True accumulations
            wu = cpool.tile([128, 512], bf16)
            nc.gpsimd.memset(wu[:], 0.0)
            for _ in range(WARMUP_MM):
                psw = pfpool.tile([RPC, D], f32, tag="psf")
                nc.tensor.matmul(psw[:], wu[:, 0:RPC], wu[:], start=True, stop=True)

            # ---- input: one big X tile, row-pair DMAs (first = immediate) --
            X = xpool.tile([128, NR, 2, D], bf16)

            def load_pair(q):
                nc.sync.dma_start(
                    X[:, 2 * q:2 * q + 2, :, :],
                    x_dram[2 * q:2 * q + 2].rearrange("r p h d -> p r h d"),
                )

            for q in range(3):
                load_pair(q)

            wtt = cpool.tile([128, 2, 128], bf16)
            nc.sync.dma_start(wtt[:], wt_dram[:])
            w9t = cpool.tile([NR, 9, RPC], bf16)
            nc.sync.dma_start(w9t[:], w9_dram[:])
            stt = cpool.tile([128, RPC], f32)
            nc.sync.dma_start(stt[:], st_dram[:])
            sft = cpool.tile([RPC, 1], f32)
            nc.sync.dma_start(sft[:], sf_dram[:])

            # strip inputs: xs cols 0..3 = j 0..3 | 4..11 = j 124..131 |
            # 12..15 = j 252..255
            xs = fpool.tile([NR, 16, D], bf16, tag="xs")
            nc.sync.dma_start(xs[:, 0:4, :], x_dram[:, 0:4, 0, :])
            nc.sync.dma_start(xs[:, 4:8, :], x_dram[:, 124:128, 0, :])
            nc.sync.dma_start(xs[:, 8:12, :], x_dram[:, 0:4, 1, :])
            nc.sync.dma_start(xs[:, 12:16, :], x_dram[:, 124:128, 1, :])

            for q in range(3, NR // 2):
                load_pair(q)

            FS = fpool.tile([RPC, 8, D], bf16, tag="FS")

            if HOLD_OUT:
                # tiny Pool op reading the GATE_PAIR-th input pair: all
                # Pool-issued out DMAs queue behind it (in-order sequencer),
                # so input transfers own the DMA engines while streaming
                gate = cpool.tile([128, 8], bf16)
                nc.gpsimd.tensor_copy(gate[:], X[:, 2 * GATE_PAIR + 1, 1, 0:8])

            # ---- main loop: 8 row quads (strip columns interleaved) --------
            # 4 rows per output DMA: SWDGE descriptor-gen cost per byte drops
            # 4x, so the drain is transfer-paced, not Pool-sequencer-paced
            for q4 in range(RPC // 4):
                i0 = 4 * q4
                for pp in (0, 1):
                    p0 = i0 + 2 * pp
                    v = tpool.tile([128, 2, 2, D], bf16, tag="v")
                    nc.vector.tensor_tensor(
                        v[:], X[:, p0 + 1:p0 + 3, :, :],
                        X[:, p0 + 3:p0 + 5, :, :], add)
                    if pp == 0:
                        ob = opool.tile([128, 4, 2, D], bf16, tag="ob")
                    for rr in (0, 1):
                        i = p0 + rr
                        ps = ppool.tile([128, 2, D], f32, tag="ps")
                        for h in (0, 1):
                            nc.tensor.matmul(ps[:, h, :], wtt[:, 1, :],
                                             v[:, rr, h, :],
                                             start=True, stop=False)
                            nc.tensor.matmul(ps[:, h, :], wtt[:, 0, :],
                                             X[:, i + 2, h, :],
                                             start=False, stop=True)
                        nc.scalar.mul(ob[:, i - i0, :, :], ps[:], stt[:, i:i + 1])
                (nc.gpsimd if HOLD_OUT else nc.scalar).dma_start(
                    y_dram[i0:i0 + 4, 2:126].rearrange("r p h d -> p r h d"),
                    ob[2:126, :, :, :],
                )
                # strip column q4: PSUM-accumulated scaled-lhsT matmuls +
                # one DVE copy applying 1/ri
                c = q4
                if 0 <= c < 8:
                    psf = pfpool.tile([RPC, D], f32, tag="psf")
                    mm = plan[c]
                    for n, (xc, s) in enumerate(mm):
                        nc.tensor.matmul(psf[:], w9t[:, s, :], xs[:, xc, :],
                                         start=(n == 0), stop=(n == len(mm) - 1))
                    nc.vector.tensor_scalar_mul(
                        FS[:, c:c + 1, :], psf[:], sft[0:RPC, 0:1])

            # fix DMAs from SP (its queue is idle by now; transfers slot in
            # behind the remaining input pairs, ahead of the output drain)
            nc.sync.dma_start(y_dram[:, 0:2, 0, :], FS[:, 0:2, :])
            nc.sync.dma_start(y_dram[:, 126:128, 0, :], FS[:, 2:4, :])
            nc.sync.dma_start(y_dram[:, 0:2, 1, :], FS[:, 4:6, :])
            nc.sync.dma_start(y_dram[:, 126:128, 1, :], FS[:, 6:8, :])
    nc.finalize()
    return nc


def _get_program():
    if "nc" not in _cache:
        _cache["nc"] = _build_nc()
        _cache["consts"] = _host_consts()
    return _cache["nc"], _cache["consts"]


def _make_in_maps(H):
    import ml_dtypes

    bf16 = ml_dtypes.bfloat16
    nc, (wt, ws9, sts, sfixes) = _get_program()
    Hb = np.asarray(H, dtype=np.float32).astype(bf16).reshape(G, G, D)
    Hp = np.zeros((G + 2 * P, G, D), dtype=bf16)
    Hp[P:P + G] = Hb
    in_maps = []
    for c in range(NC):
        shard = Hp[RPC * c: RPC * c + NR]                      # [36, 256, 512]
        shard = shard.reshape(NR, 2, 128, D).transpose(0, 2, 1, 3)
        shard = np.ascontiguousarray(shard)                    # [36, 128, 2, 512]
        in_maps.append(
            {"x": shard, "wt": wt, "ws9": ws9,
             "st": sts[c], "sfix": sfixes[c]}
        )
    return in_maps


def kernel(H, xy=None):
    from concourse.bass_utils import run_bass_kernel_spmd

    nc, _ = _get_program()
    in_maps = _make_in_maps(H)
    res = run_bass_kernel_spmd(nc, in_maps, list(range(NC))).results
    # y [32, 128, 2, 512] bf16 -> [32*256, 512] f32 with j = h*128 + p
    outs = []
    for c in range(NC):
        y = np.asarray(res[c]["y"]).astype(np.float32)
        outs.append(y.transpose(0, 2, 1, 3).reshape(RPC * G, D))
    return np.concatenate(outs, axis=0)


# revision 34
# speedup vs baseline: 1.0816x; 1.0445x over previous
"""Localized embedding layer (separable 5x5 Gaussian stencil) on 8 trn2 cores.

Math: out[i,j,:] = sum_{di,dj} w(di)w(dj) H[i+di,j+dj,:] / (ri(i)*rj(j))
with w(d) = exp(-c*d^2), c = TILE^2/(2 sigma^2); ri/rj = sums of the valid
taps actually applied (the +-2 i-taps, weight w2 ~ 4.4e-5, are dropped — a
~1e-4 relative contribution, far below bf16 rounding noise).

v3 design (bf16 end-to-end, DMA-roofline oriented):
  - All HBM traffic in bf16 (host converts): ~18 MB/core vs 36 MB f32.
  - Input per core: one SBUF tile X[128p(j%128), 36 row, 2 half, 512 d],
    loaded as 18 row-pair DMAs (SP sequencer).
  - i-conv: v = x[i+1]+x[i+3] as paired DVE tensor_tensor (bf16 2x mode);
    tap weights folded into the j-conv matmuls.
  - j-conv: 2 PSUM-accumulated bf16 matmuls per half with Toeplitz lhsT:
    ps = w1*T@v + T@x[i+2];  T[m,k] = w(m-k)/w_full.
  - Act engine: PSUM->SBUF copy with per-row scale 1/ri, bf16 out.
  - Out DMAs (row quads) issued from Pool (SWDGE), all gated behind a
    dummy Pool op that depends on the last input pair: input transfers own
    the DMA engines first (no in/out interleaving), then outputs drain
    back-to-back from deep (OB_BUFS row-quad) SBUF buffering.
  - Columns j in {0,1,126..129,254,255} (grid edge + half-boundary) are
    recomputed by a strip pass: per output column, 3-5 PSUM-accumulated
    matmuls with scaled i-conv lhsT (j-comb folded into the lhsT scale —
    convolutions commute), one DVE copy applying 1/ri, 4 small SP DMAs.
    Strip column c is emitted inside main-loop quad c.
"""

import sys
import numpy as np

if "/opt/trn_rl_repo" not in sys.path:
    sys.path.insert(0, "/opt/trn_rl_repo")

G = 256          # grid side
D = 512          # feature dim
P = 2            # grid_step halo
NC = 8           # cores
RPC = G // NC    # rows per core = 32
NR = RPC + 2      # input rows per core = 34 (halo +-1: the
                  # +-2 i-taps are dropped, see _r_vec)
TILE = 448.0
SIGMA = 200.0

_cache = {}

# tuning knobs (A/B tested via TimelineSim)
OB_BUFS = 6        # output row-quad tiles buffered in SBUF
HOLD_OUT = True    # gate out DMAs behind the GATE_PAIR-th input pair
WARMUP_MM = 14     # dummy PE matmuls to ramp the p-state at kernel start


def _weights():
    c = TILE * TILE / (2.0 * SIGMA * SIGMA)
    return np.exp(-c * np.arange(-P, P + 1) ** 2)   # [w2,w1,1,w1,w2] f64


def _r_vec(drop2=False):
    """r(i) = sum of valid 1D taps at row i (same for columns).

    drop2: exclude the +-2 taps — used for the i-dimension, where the kernel
    skips those taps; the normalizer must match the taps actually applied.
    """
    w = _weights()
    r = np.zeros(G)
    for d in range(-P, P + 1):
        if drop2 and abs(d) == 2:
            continue
        lo, hi = max(0, -d), min(G, G - d)
        r[lo:hi] += w[d + P]
    return r


def _strip_plan():
    """Per fix column c (FS order [j0,j1,j126..j129,j254,j255]): list of
    (xs_col, lhsT_idx). lhsT_idx = 3*denom_group + |d|, denom group
    0 -> w_full, 1 -> r(0), 2 -> r(1). xs cols: 0..3 = j 0..3,
    4..11 = j 124..131, 12..15 = j 252..255."""
    plan = [
        [(0, 3 + 0), (1, 3 + 1), (2, 3 + 2)],                       # j0   /r0
        [(0, 6 + 1), (1, 6 + 0), (2, 6 + 1), (3, 6 + 2)],           # j1   /r1
        [(4, 2), (5, 1), (6, 0), (7, 1), (8, 2)],                   # j126 /wf
        [(5, 2), (6, 1), (7, 0), (8, 1), (9, 2)],                   # j127 /wf
        [(6, 2), (7, 1), (8, 0), (9, 1), (10, 2)],                  # j128 /wf
        [(7, 2), (8, 1), (9, 0), (10, 1), (11, 2)],                 # j129 /wf
        [(12, 6 + 2), (13, 6 + 1), (14, 6 + 0), (15, 6 + 1)],       # j254 /r1
        [(13, 3 + 2), (14, 3 + 1), (15, 3 + 0)],                    # j255 /r0
    ]
    return plan


def _host_consts():
    import ml_dtypes

    bf16 = ml_dtypes.bfloat16
    w = _weights()
    ri = _r_vec(drop2=True)      # i-dim normalizer
    rj = _r_vec()                # j-dim normalizer (all 5 taps)
    w_full = w.sum()
    w1 = float(w[1])
    # Toeplitz block T[m, k] = w(m-k)/w_full, |m-k| <= 2 (interior j columns;
    # edge/boundary columns are recomputed by the strip pass). lhsT = T.
    T = np.zeros((128, 128))
    for d in range(-P, P + 1):
        for m in range(128):
            k = m + d
            if 0 <= k < 128:
                T[k, m] = w[d + P] / w_full
    wt = np.zeros((128, 2, 128), dtype=np.float64)
    wt[:, 0, :] = T
    wt[:, 1, :] = w1 * T
    wt = wt.astype(bf16)
    # strip i-conv lhsT base [36, 32]: (Wb @ xs_col)[i] = sum_{k=1..3} w[k]
    # * xs[i+k]; 9 scaled variants fold the j-comb weight / column norm in.
    Wb = np.zeros((NR, RPC))
    for i in range(RPC):
        for k in range(3):
            Wb[i + k, i] = w[k + 1]
    denoms = [w_full, rj[0], rj[1]]
    taps = [w[2], w[3], w[4]]    # w0, w1, w2
    ws9 = np.zeros((NR, 9, RPC), dtype=np.float64)
    for g in range(3):
        for t in range(3):
            ws9[:, 3 * g + t, :] = (taps[t] / denoms[g]) * Wb
    ws9 = ws9.astype(bf16)
    # per-core row scales: st[:, i] = 1/ri(global_row); sfix same, [32,1]
    sts, sfixes = [], []
    for c in range(NC):
        s = (1.0 / ri[RPC * c: RPC * (c + 1)]).astype(np.float32)
        sts.append(np.broadcast_to(s[None, :], (128, RPC)).copy())
        sf = np.zeros((RPC, 1), dtype=np.float32)
        sf[:, 0] = s
        sfixes.append(sf)
    return wt, ws9, sts, sfixes


def _build_nc():
    import concourse.bass as bass  # noqa: F401
    import concourse.mybir as mybir
    import concourse.tile as tile
    from concourse import bacc

    f32 = mybir.dt.float32
    bf16 = mybir.dt.bfloat16
    add = mybir.AluOpType.add

    nc = bacc.Bacc(None, target_bir_lowering=False, debug=False)
    x_dram = nc.declare_dram_parameter("x", [NR, 128, 2, D], bf16, isOutput=False)
    wt_dram = nc.declare_dram_parameter("wt", [128, 2, 128], bf16, isOutput=False)
    w9_dram = nc.declare_dram_parameter("ws9", [NR, 9, RPC], bf16, isOutput=False)
    xs_dram = nc.declare_dram_parameter("xsd", [NR, 16, D], bf16, isOutput=False)
    st_dram = nc.declare_dram_parameter("st", [128, RPC], f32, isOutput=False)
    sf_dram = nc.declare_dram_parameter("sfix", [RPC, 1], f32, isOutput=False)
    y_dram = nc.declare_dram_parameter("y", [RPC, 128, 2, D], bf16, isOutput=True)

    plan = _strip_plan()

    with tile.TileContext(nc) as tc:
        with (
            tc.tile_pool(name="const", bufs=1) as cpool,
            tc.tile_pool(name="x", bufs=1) as xpool,
            tc.tile_pool(name="uv", bufs=3) as tpool,
            tc.tile_pool(name="out", bufs=OB_BUFS) as opool,
            tc.tile_pool(name="fix", bufs=1) as fpool,
            tc.tile_pool(name="psum", bufs=3, space="PSUM") as ppool,
            tc.tile_pool(name="psfix", bufs=2, space="PSUM") as pfpool,
        ):
            # ---- PE warm-up: dummy matmuls on an uninitialized tile (no
            # input deps -> run at t~0 back-to-back) ramp the tensor engine
            # to full clock before the first real matmul arrives; their
            # garbage PSUM output is overwritten by start=True accumulations
            wu = cpool.tile([128, 512], bf16)
            nc.gpsimd.memset(wu[:], 0.0)
            for _ in range(WARMUP_MM):
                psw = pfpool.tile([RPC, D], f32, tag="psf")
                nc.tensor.matmul(psw[:], wu[:, 0:RPC], wu[:], start=True, stop=True)

            # ---- input: one big X tile, row-pair DMAs (first = immediate) --
            X = xpool.tile([128, NR, 2, D], bf16)

            def load_pair(q):
                nc.sync.dma_start(
                    X[:, 2 * q:2 * q + 2, :, :],
                    x_dram[2 * q:2 * q + 2].rearrange("r p h d -> p r h d"),
                )

            for q in range(3):
                load_pair(q)

            wtt = cpool.tile([128, 2, 128], bf16)
            nc.sync.dma_start(wtt[:], wt_dram[:])
            w9t = cpool.tile([NR, 9, RPC], bf16)
            nc.sync.dma_start(w9t[:], w9_dram[:])
            stt = cpool.tile([128, RPC], f32)
            nc.sync.dma_start(stt[:], st_dram[:])
            sft = cpool.tile([RPC, 1], f32)
            nc.sync.dma_start(sft[:], sf_dram[:])

            # strip inputs: xs cols 0..3 = j 0..3 | 4..11 = j 124..131 |
            # 12..15 = j 252..255
            xs = fpool.tile([NR, 16, D], bf16, tag="xs")
            nc.sync.dma_start(xs[:], xs_dram[:])

            for q in range(3, NR // 2):
                load_pair(q)

            FS = fpool.tile([RPC, 8, D], bf16, tag="FS")

            if HOLD_OUT:
                # tiny Pool op reading the GATE_PAIR-th input pair: all
                # Pool-issued out DMAs queue behind it (in-order sequencer),
                # so input transfers own the DMA engines while streaming
                gate = cpool.tile([128, 8], bf16)
                nc.gpsimd.tensor_copy(gate[:], X[:, NR - 1, 1, 0:8])

            # ---- main loop: 8 row quads (strip columns interleaved) --------
            # 4 rows per output DMA: SWDGE descriptor-gen cost per byte drops
            # 4x, so the drain is transfer-paced, not Pool-sequencer-paced
            for q4 in range(RPC // 4):
                i0 = 4 * q4
                for pp in (0, 1):
                    p0 = i0 + 2 * pp
                    v = tpool.tile([128, 2, 2, D], bf16, tag="v")
                    nc.vector.tensor_tensor(
                        v[:], X[:, p0:p0 + 2, :, :],
                        X[:, p0 + 2:p0 + 4, :, :], add)
                    if pp == 0:
                        ob = opool.tile([128, 4, 2, D], bf16, tag="ob")
                    for rr in (0, 1):
                        i = p0 + rr
                        ps = ppool.tile([128, 2, D], f32, tag="ps")
                        for h in (0, 1):
                            nc.tensor.matmul(ps[:, h, :], wtt[:, 1, :],
                                             v[:, rr, h, :],
                                             start=True, stop=False)
                            nc.tensor.matmul(ps[:, h, :], wtt[:, 0, :],
                                             X[:, i + 1, h, :],
                                             start=False, stop=True)
                        nc.scalar.mul(ob[:, i - i0, :, :], ps[:], stt[:, i:i + 1])
                (nc.gpsimd if HOLD_OUT else nc.scalar).dma_start(
                    y_dram[i0:i0 + 4, 2:126].rearrange("r p h d -> p r h d"),
                    ob[2:126, :, :, :],
                )
                # strip column q4: PSUM-accumulated scaled-lhsT matmuls +
                # one DVE copy applying 1/ri
                c = q4
                if 0 <= c < 8:
                    psf = pfpool.tile([RPC, D], f32, tag="psf")
                    mm = plan[c]
                    for n, (xc, s) in enumerate(mm):
                        nc.tensor.matmul(psf[:], w9t[:, s, :], xs[:, xc, :],
                                         start=(n == 0), stop=(n == len(mm) - 1))
                    nc.vector.tensor_scalar_mul(
                        FS[:, c:c + 1, :], psf[:], sft[0:RPC, 0:1])

            # fix DMAs from SP (its queue is idle by now; transfers slot in
            # behind the remaining input pairs, ahead of the output drain)
            nc.sync.dma_start(y_dram[:, 0:2, 0, :], FS[:, 0:2, :])
            nc.sync.dma_start(y_dram[:, 126:128, 0, :], FS[:, 2:4, :])
            nc.sync.dma_start(y_dram[:, 0:2, 1, :], FS[:, 4:6, :])
            nc.sync.dma_start(y_dram[:, 126:128, 1, :], FS[:, 6:8, :])
    nc.finalize()
    return nc


def _get_program():
    if "nc" not in _cache:
        _cache["nc"] = _build_nc()
        _cache["consts"] = _host_consts()
    return _cache["nc"], _cache["consts"]


def _make_in_maps(H):
    import ml_dtypes

    bf16 = ml_dtypes.bfloat16
    nc, (wt, ws9, sts, sfixes) = _get_program()
    Hb = np.asarray(H, dtype=np.float32).astype(bf16).reshape(G, G, D)
    Hp = np.zeros((G + 2 * P, G, D), dtype=bf16)
    Hp[P:P + G] = Hb
    in_maps = []
    for c in range(NC):
        raw = Hp[RPC * c + 1: RPC * c + 1 + NR]                # [34, 256, 512]
        shard = raw.reshape(NR, 2, 128, D).transpose(0, 2, 1, 3)
        shard = np.ascontiguousarray(shard)                    # [34, 128, 2, 512]
        # strip columns j in {0..3, 124..131, 252..255}, host-gathered
        xsd = np.ascontiguousarray(
            raw[:, [0, 1, 2, 3, 124, 125, 126, 127,
                    128, 129, 130, 131, 252, 253, 254, 255], :])
        in_maps.append(
            {"x": shard, "xsd": xsd, "wt": wt, "ws9": ws9,
             "st": sts[c], "sfix": sfixes[c]}
        )
    return in_maps


def kernel(H, xy=None):
    from concourse.bass_utils import run_bass_kernel_spmd

    nc, _ = _get_program()
    in_maps = _make_in_maps(H)
    res = run_bass_kernel_spmd(nc, in_maps, list(range(NC))).results
    # y [32, 128, 2, 512] bf16 -> [32*256, 512] f32 with j = h*128 + p
    outs = []
    for c in range(NC):
        y = np.asarray(res[c]["y"]).astype(np.float32)
        outs.append(y.transpose(0, 2, 1, 3).reshape(RPC * G, D))
    return np.concatenate(outs, axis=0)
